# revision 7
# baseline (speedup 1.0000x reference)
"""Causal single-head self-attention (B=8, S=1024, D=1024, f32) on 8 TRN2 cores.

Sharding: data-parallel over batch (1 batch element per core); the four
d_model^2 weights are replicated. Host-side prep transposes x[b] -> xT [d, s]
and each weight -> wT [d, e] so every on-chip matmul contracts over the
partition dimension with no on-chip transposes.

Per-core dataflow (S=1024 rows of one batch element):
  qT[e,s] = wqT.T @ xT        kT[e,s] = wkT.T @ xT       v[s,e] = xT.T @ wvT
  scoresT[j,i] = kT.T @ qT    (only causal-needed 128x512 blocks)
  attnT = exp(scoresT/32)     (ACT engine, reads PSUM; causal mask via
                               affine_select on diagonal-crossing blocks)
  r[i]  = sum_j attnT[j,i]    (matmuls with a ones vector -> [i,1] PSUM)
  outT[d,i] = v.T @ attnT
  y[s,e] = outT.T @ woT, then y[s,:] *= 1/r[s] fused into the PSUM->SBUF copy.
"""

import sys

sys.path.insert(0, "/opt/trn_rl_repo")

from contextlib import ExitStack

import numpy as np

import concourse.bass as bass
from concourse import bacc
import concourse.mybir as mybir
import concourse.tile as tile
from concourse.bass_utils import run_bass_kernel_spmd

B, S, D = 8, 1024, 1024
P = 128          # partition / stationary tile size
NB = 512         # moving-operand block (max for 4-byte dtypes, = 1 PSUM bank)
NT = S // P      # 8 tiles of 128 along s/d/e/j
NBLK = S // NB   # 2 blocks of 512 along s/i/e
SCALE = 1.0 / np.sqrt(float(D))

F32 = mybir.dt.float32
MM_DT = mybir.dt.float32r  # fp32r: full-rate (1 cyc/row) matmul at N>=256

N_CORES = 8

LAST_RESULTS = None  # BassKernelResults of the most recent run (for test.py)


def _build():
    nc = bacc.Bacc("TRN2", target_bir_lowering=False, debug=False)

    xT_d = nc.dram_tensor("xT", [D, S], F32, kind="ExternalInput").ap()
    wqT_d = nc.dram_tensor("wqT", [D, D], F32, kind="ExternalInput").ap()
    wkT_d = nc.dram_tensor("wkT", [D, D], F32, kind="ExternalInput").ap()
    wvT_d = nc.dram_tensor("wvT", [D, D], F32, kind="ExternalInput").ap()
    woT_d = nc.dram_tensor("woT", [D, D], F32, kind="ExternalInput").ap()
    y_d = nc.dram_tensor("y", [S, D], F32, kind="ExternalOutput").ap()

    # DRAM [1024, 1024] -> SBUF [128, 8192]: column range t*1024..(t+1)*1024 of
    # the SBUF view holds rows t*128..(t+1)*128 of the DRAM matrix. Both sides
    # expressed as 3D APs [p, t, cols] for a single DMA.
    def big_load(sbuf_tile, dram_ap):
        eng = nc.sync if sbuf_tile.dtype == dram_ap.dtype else nc.gpsimd
        eng.dma_start(
            sbuf_tile.rearrange("p (t s) -> p t s", t=NT),
            dram_ap.rearrange("(t p) s -> p t s", p=P),
        )

    with tile.TileContext(nc) as tc, ExitStack() as ctx:
        consts = ctx.enter_context(tc.tile_pool(name="consts", bufs=1))
        ones_f32 = consts.tile([P, 8], F32)
        nc.gpsimd.memset(ones_f32, 1.0)
        ones = consts.tile([P, 8], MM_DT)
        nc.vector.tensor_copy(out=ones, in_=ones_f32)
        zbias = consts.tile([P, 1], F32)
        nc.gpsimd.memset(zbias, 0.0)

        psum = ctx.enter_context(tc.tile_pool(name="psum", bufs=6, space="PSUM"))

        # Two weight slots; wv reuses wq's slot, wo reuses wk's (WAR deps make
        # the DMAs wait for the previous phase's matmuls automatically).
        wpool = ctx.enter_context(tc.tile_pool(name="wpool", bufs=2))
        qpool = ctx.enter_context(tc.tile_pool(name="qpool", bufs=1))
        kpool = ctx.enter_context(tc.tile_pool(name="kpool", bufs=1))
        vpool = ctx.enter_context(tc.tile_pool(name="vpool", bufs=1))

        qT = qpool.tile([P, NT * S], MM_DT, name="qT")
        kT = kpool.tile([P, NT * S], MM_DT, name="kT")
        v = vpool.tile([P, NT * S], MM_DT, name="v")

        wq = wpool.tile([P, NT * D], MM_DT, tag="w", name="wq")
        wk = wpool.tile([P, NT * D], MM_DT, tag="w", name="wk")
        big_load(wq, wqT_d)
        big_load(wk, wkT_d)

        def proj_to_T(w_sb, dst):
            # dst[e, s] = w.T @ xT : stationary w[d, e128], moving xT[d, s512]
            for et in range(NT):
                for sb in range(NBLK):
                    pt = psum.tile([P, NB], F32, tag="mm", bufs=6, name="pt")
                    for dt in range(NT):
                        nc.tensor.matmul(
                            pt,
                            w_sb[:, dt * D + et * P : dt * D + (et + 1) * P],
                            xsb[:, dt * S + sb * NB : dt * S + (sb + 1) * NB],
                            start=(dt == 0),
                            stop=(dt == NT - 1),
                        )
                    nc.vector.tensor_copy(
                        out=dst[:, et * S + sb * NB : et * S + (sb + 1) * NB],
                        in_=pt,
                    )

        with tc.tile_pool(name="xpool", bufs=1) as xpool:
            xsb = xpool.tile([P, NT * S], MM_DT, name="xsb")
            big_load(xsb, xT_d)

            proj_to_T(wq, qT)   # qT[e, s]
            proj_to_T(wk, kT)   # kT[e, s]

            wv = wpool.tile([P, NT * D], MM_DT, tag="w", name="wv")
            big_load(wv, wvT_d)

            # v[s, e] natural: stationary xT[d, s128], moving wvT[d, e512]
            for st in range(NT):
                for eb in range(NBLK):
                    pt = psum.tile([P, NB], F32, tag="mm", bufs=6, name="pt")
                    for dt in range(NT):
                        nc.tensor.matmul(
                            pt,
                            xsb[:, dt * S + st * P : dt * S + (st + 1) * P],
                            wv[:, dt * D + eb * NB : dt * D + (eb + 1) * NB],
                            start=(dt == 0),
                            stop=(dt == NT - 1),
                        )
                    nc.vector.tensor_copy(
                        out=v[:, st * D + eb * NB : st * D + (eb + 1) * NB],
                        in_=pt,
                    )

        wo = wpool.tile([P, NT * D], MM_DT, tag="w", name="wo")
        big_load(wo, woT_d)

        apool = ctx.enter_context(tc.tile_pool(name="apool", bufs=9))
        opool = ctx.enter_context(tc.tile_pool(name="opool", bufs=8))
        ypool = ctx.enter_context(tc.tile_pool(name="ypool", bufs=2))
        rpool = ctx.enter_context(tc.tile_pool(name="rpool", bufs=6))

        for ib in range(NBLK):
            jt_max = (ib + 1) * (NB // P)  # causal: j-tiles 0..jt_max-1

            # scoresT[j, i] -> exp -> attnT tiles in SBUF
            attnT = []
            for jt in range(jt_max):
                ps = psum.tile([P, NB], F32, tag="mm", bufs=6, name="ps")
                for et in range(NT):
                    nc.tensor.matmul(
                        ps,
                        kT[:, et * S + jt * P : et * S + (jt + 1) * P],
                        qT[:, et * S + ib * NB : et * S + (ib + 1) * NB],
                        start=(et == 0),
                        stop=(et == NT - 1),
                    )
                at = apool.tile([P, NB], MM_DT, tag="attn", name="at")
                nc.scalar.activation(
                    out=at,
                    in_=ps,
                    func=mybir.ActivationFunctionType.Exp,
                    bias=zbias,
                    scale=SCALE,
                )
                # blocks fully below the diagonal need no mask
                if jt * P + P - 1 > ib * NB:
                    # keep where i_global - j_global >= 0, else 0
                    nc.gpsimd.affine_select(
                        out=at,
                        in_=at,
                        compare_op=mybir.AluOpType.is_ge,
                        fill=0.0,
                        base=ib * NB - jt * P,
                        pattern=[[1, NB]],
                        channel_multiplier=-1,
                    )
                attnT.append(at)

            # softmax denominators r[i] as [i128, 1], then reciprocal
            recips = []
            for st in range(NB // P):
                it = ib * (NB // P) + st  # global i-tile index
                pr = psum.tile([P, 8], F32, tag="rs", bufs=2, name="pr")
                for jt in range(it + 1):
                    nc.tensor.matmul(
                        pr,
                        attnT[jt][:, st * P : (st + 1) * P],
                        ones,
                        start=(jt == 0),
                        stop=(jt == it),
                    )
                rc = rpool.tile([P, 1], F32, tag="rc", bufs=6, name="rc")
                nc.vector.tensor_copy(out=rc, in_=pr[:, 0:1])
                nc.vector.reciprocal(out=rc, in_=rc)
                recips.append(rc)

            # outT[d, i] = v.T @ attnT
            outT = []
            for dt in range(NT):
                po = psum.tile([P, NB], F32, tag="mm", bufs=6, name="po")
                for jt in range(jt_max):
                    nc.tensor.matmul(
                        po,
                        v[:, jt * D + dt * P : jt * D + (dt + 1) * P],
                        attnT[jt],
                        start=(jt == 0),
                        stop=(jt == jt_max - 1),
                    )
                ot = opool.tile([P, NB], MM_DT, tag="ot", name="ot")
                nc.vector.tensor_copy(out=ot, in_=po)
                outT.append(ot)

            # y[s, e] = outT.T @ woT with softmax normalization fused in
            for st in range(NB // P):
                ysb = ypool.tile([P, D], F32, tag="y", name="ysb")
                for eb in range(NBLK):
                    py = psum.tile([P, NB], F32, tag="mm", bufs=6, name="py")
                    for dt in range(NT):
                        nc.tensor.matmul(
                            py,
                            outT[dt][:, st * P : (st + 1) * P],
                            wo[:, dt * D + eb * NB : dt * D + (eb + 1) * NB],
                            start=(dt == 0),
                            stop=(dt == NT - 1),
                        )
                    nc.vector.tensor_scalar_mul(
                        ysb[:, eb * NB : (eb + 1) * NB], py, recips[st]
                    )
                row0 = (ib * (NB // P) + st) * P
                nc.sync.dma_start(y_d[row0 : row0 + P, :], ysb)

    nc.finalize()
    return nc


_CACHED_NC = None


def kernel(x, wq, wk, wv, wo, _trace=False):
    global LAST_RESULTS, _CACHED_NC
    assert x.shape == (B, S, D)
    if _CACHED_NC is None:
        _CACHED_NC = _build()
    nc = _CACHED_NC

    wqT = np.ascontiguousarray(wq.T)
    wkT = np.ascontiguousarray(wk.T)
    wvT = np.ascontiguousarray(wv.T)
    woT = np.ascontiguousarray(wo.T)
    in_maps = [
        {
            "xT": np.ascontiguousarray(x[b].T),
            "wqT": wqT,
            "wkT": wkT,
            "wvT": wvT,
            "woT": woT,
        }
        for b in range(N_CORES)
    ]

    res = run_bass_kernel_spmd(
        nc, in_maps, core_ids=list(range(N_CORES)), trace=_trace
    )
    LAST_RESULTS = res
    out = np.stack([res.results[b]["y"] for b in range(N_CORES)], axis=0)
    return out.astype(np.float32, copy=False)


# revision 8
# speedup vs baseline: 1.0978x; 1.0978x over previous
"""Causal single-head self-attention (B=8, S=1024, D=1024, f32) on 8 TRN2 cores.

Sharding: data-parallel over batch (1 batch element per core); the four
d_model^2 weights are replicated. Host-side prep transposes x[b] -> xT [d, s]
and each weight -> wT [d, e] so every on-chip matmul contracts over the
partition dimension with no on-chip transposes.

Per-core dataflow (S=1024 rows of one batch element):
  qT[e,s] = wqT.T @ xT        kT[e,s] = wkT.T @ xT       v[s,e] = xT.T @ wvT
  scoresT[j,i] = kT.T @ qT    (only causal-needed 128x512 blocks)
  attnT = exp(scoresT/32)     (ACT engine, reads PSUM; causal mask via
                               affine_select on diagonal-crossing blocks)
  r[i]  = sum_j attnT[j,i]    (matmuls with a ones vector -> [i,1] PSUM)
  outT[d,i] = v.T @ attnT
  y[s,e] = outT.T @ woT, then y[s,:] *= 1/r[s] fused into the PSUM->SBUF copy.

Matmuls run in float32r (full-rate 4-byte mode, TF32-class rounding). Input
DRAM tensors are declared float32r directly (same bits as f32) so plain HWDGE
DMAs satisfy the BIR verifier's fp32r rounding rule. The first projection
phase is DMA-paced: x arrives as 8 row-slabs interleaved with wq column
chunks, and the first 6 PSUM groups accumulate d-tile-major so matmuls chase
the arriving slabs instead of stalling for the full 8 MB.
"""

import sys

sys.path.insert(0, "/opt/trn_rl_repo")

from contextlib import ExitStack

import numpy as np

import concourse.bass as bass
from concourse import bacc
import concourse.mybir as mybir
import concourse.tile as tile
from concourse.tile import add_dep_helper
from concourse.bass_utils import run_bass_kernel_spmd

B, S, D = 8, 1024, 1024
P = 128          # partition / stationary tile size
NB = 512         # moving-operand block (max for 4-byte dtypes, = 1 PSUM bank)
NT = S // P      # 8 tiles of 128 along s/d/e/j
NBLK = S // NB   # 2 blocks of 512 along s/i/e
SCALE = 1.0 / np.sqrt(float(D))

F32 = mybir.dt.float32
MM_DT = mybir.dt.float32r  # fp32r: full-rate (1 cyc/row) matmul at N>=256

N_CORES = 8

LAST_RESULTS = None  # BassKernelResults of the most recent run (for test.py)


def _build():
    nc = bacc.Bacc("TRN2", target_bir_lowering=False, debug=False)

    xT_d = nc.dram_tensor("xT", [D, S], MM_DT, kind="ExternalInput").ap()
    wqT_d = nc.dram_tensor("wqT", [D, D], MM_DT, kind="ExternalInput").ap()
    wkT_d = nc.dram_tensor("wkT", [D, D], MM_DT, kind="ExternalInput").ap()
    wvT_d = nc.dram_tensor("wvT", [D, D], MM_DT, kind="ExternalInput").ap()
    woT_d = nc.dram_tensor("woT", [D, D], MM_DT, kind="ExternalInput").ap()
    y_d = nc.dram_tensor("y", [S, D], F32, kind="ExternalOutput").ap()

    # SBUF layout of a transposed 1024x1024 matrix: big tile [128, 8192] where
    # column range t*1024..(t+1)*1024 holds DRAM rows t*128..(t+1)*128.
    def slab_load(sbuf_tile, dram_ap, t):
        # one row-slab: DRAM rows t*128..(t+1)*128 (512 KB contiguous)
        return nc.sync.dma_start(
            sbuf_tile[:, t * S : (t + 1) * S],
            dram_ap[t * P : (t + 1) * P, :],
        )

    def chunk_load(sbuf_tile, dram_ap, c):
        # one column-chunk: DRAM cols c*128..(c+1)*128 across all row-slabs
        return nc.sync.dma_start(
            sbuf_tile.rearrange("p (t e) -> p t e", t=NT)[:, :, c * P : (c + 1) * P],
            dram_ap.rearrange("(t p) e -> p t e", p=P)[:, :, c * P : (c + 1) * P],
        )

    with tile.TileContext(nc) as tc, ExitStack() as ctx:
        consts = ctx.enter_context(tc.tile_pool(name="consts", bufs=1))
        ones_f32 = consts.tile([P, 8], F32)
        nc.gpsimd.memset(ones_f32, 1.0)
        ones = consts.tile([P, 8], MM_DT)
        nc.vector.tensor_copy(out=ones, in_=ones_f32)
        zbias = consts.tile([P, 1], F32)
        nc.gpsimd.memset(zbias, 0.0)

        psum = ctx.enter_context(tc.tile_pool(name="psum", bufs=6, space="PSUM"))

        # Two weight slots; wv reuses wq's slot, wo reuses wk's (WAR deps make
        # the DMAs wait for the previous phase's matmuls automatically).
        wpool = ctx.enter_context(tc.tile_pool(name="wpool", bufs=2))
        qpool = ctx.enter_context(tc.tile_pool(name="qpool", bufs=1))
        kpool = ctx.enter_context(tc.tile_pool(name="kpool", bufs=1))
        vpool = ctx.enter_context(tc.tile_pool(name="vpool", bufs=1))

        qT = qpool.tile([P, NT * S], MM_DT, name="qT")
        kT = kpool.tile([P, NT * S], MM_DT, name="kT")
        v = vpool.tile([P, NT * S], MM_DT, name="v")

        wq = wpool.tile([P, NT * D], MM_DT, tag="w", name="wq")
        wk = wpool.tile([P, NT * D], MM_DT, tag="w", name="wk")

        with tc.tile_pool(name="xpool", bufs=1) as xpool:
            xsb = xpool.tile([P, NT * S], MM_DT, name="xsb")

            # Interleave x row-slabs with wq column-chunks so the first PSUM
            # groups (which need all of x but only wq chunk c per e-tile c)
            # unblock as early as possible.
            for t in range(NT):
                slab_load(xsb, xT_d, t)
                chunk_load(wq, wqT_d, t)

            def mm_q(pt, et, sb, dt):
                nc.tensor.matmul(
                    pt,
                    wq[:, dt * D + et * P : dt * D + (et + 1) * P],
                    xsb[:, dt * S + sb * NB : dt * S + (sb + 1) * NB],
                    start=(dt == 0),
                    stop=(dt == NT - 1),
                )

            q_copies = {}  # (et, sb) -> copy instruction (for wk prefetch deps)

            def q_copy(pt, et, sb):
                inst = nc.vector.tensor_copy(
                    out=qT[:, et * S + sb * NB : et * S + (sb + 1) * NB],
                    in_=pt,
                )
                q_copies[(et, sb)] = inst
                return inst

            # Phase 0 of P_q: 6 PSUM groups accumulated d-tile-major so the
            # matmul stream follows the arriving x slabs.
            groups = [(et, sb) for et in range(3) for sb in range(NBLK)]
            pts = {}
            for g in groups:
                pts[g] = psum.tile([P, NB], F32, tag="mm", bufs=6, name="pt")
            for dt in range(NT):
                for (et, sb) in groups:
                    mm_q(pts[(et, sb)], et, sb, dt)
            for (et, sb) in groups:
                q_copy(pts[(et, sb)], et, sb)

            # Remaining e-tiles of P_q, standard order.
            for et in range(3, NT):
                for sb in range(NBLK):
                    pt = psum.tile([P, NB], F32, tag="mm", bufs=6, name="pt")
                    for dt in range(NT):
                        mm_q(pt, et, sb, dt)
                    q_copy(pt, et, sb)

            # wk slabs prefetch spread across P_q so they don't steal DMA
            # bandwidth from the x/wq ramp.
            for t in range(NT):
                dma = slab_load(wk, wkT_d, t)
                anchor = q_copies.get((min(1 + t // 2, NT - 1), t % 2))
                if anchor is not None:
                    add_dep_helper(dma.ins, anchor.ins, reason="wk prefetch pacing")

            # P_k: kT[e, s], all inputs resident by now.
            for et in range(NT):
                for sb in range(NBLK):
                    pt = psum.tile([P, NB], F32, tag="mm", bufs=6, name="pt")
                    for dt in range(NT):
                        nc.tensor.matmul(
                            pt,
                            wk[:, dt * D + et * P : dt * D + (et + 1) * P],
                            xsb[:, dt * S + sb * NB : dt * S + (sb + 1) * NB],
                            start=(dt == 0),
                            stop=(dt == NT - 1),
                        )
                    nc.vector.tensor_copy(
                        out=kT[:, et * S + sb * NB : et * S + (sb + 1) * NB],
                        in_=pt,
                    )

            wv = wpool.tile([P, NT * D], MM_DT, tag="w", name="wv")
            for t in range(NT):
                slab_load(wv, wvT_d, t)

            # P_v: v[s, e] natural: stationary xT[d, s128], moving wvT[d, e512]
            for st in range(NT):
                for eb in range(NBLK):
                    pt = psum.tile([P, NB], F32, tag="mm", bufs=6, name="pt")
                    for dt in range(NT):
                        nc.tensor.matmul(
                            pt,
                            xsb[:, dt * S + st * P : dt * S + (st + 1) * P],
                            wv[:, dt * D + eb * NB : dt * D + (eb + 1) * NB],
                            start=(dt == 0),
                            stop=(dt == NT - 1),
                        )
                    nc.vector.tensor_copy(
                        out=v[:, st * D + eb * NB : st * D + (eb + 1) * NB],
                        in_=pt,
                    )

        wo = wpool.tile([P, NT * D], MM_DT, tag="w", name="wo")
        for t in range(NT):
            slab_load(wo, woT_d, t)

        apool = ctx.enter_context(tc.tile_pool(name="apool", bufs=10))
        opool = ctx.enter_context(tc.tile_pool(name="opool", bufs=8))
        ypool = ctx.enter_context(tc.tile_pool(name="ypool", bufs=2))
        rpool = ctx.enter_context(tc.tile_pool(name="rpool", bufs=6))

        for ib in range(NBLK):
            jt_max = (ib + 1) * (NB // P)  # causal: j-tiles 0..jt_max-1

            # scoresT[j, i] -> exp -> attnT tiles in SBUF
            attnT = []
            for jt in range(jt_max):
                ps = psum.tile([P, NB], F32, tag="mm", bufs=6, name="ps")
                for et in range(NT):
                    nc.tensor.matmul(
                        ps,
                        kT[:, et * S + jt * P : et * S + (jt + 1) * P],
                        qT[:, et * S + ib * NB : et * S + (ib + 1) * NB],
                        start=(et == 0),
                        stop=(et == NT - 1),
                    )
                at = apool.tile([P, NB], MM_DT, tag="attn", name="at")
                nc.scalar.activation(
                    out=at,
                    in_=ps,
                    func=mybir.ActivationFunctionType.Exp,
                    bias=zbias,
                    scale=SCALE,
                )
                # blocks fully below the diagonal need no mask
                if jt * P + P - 1 > ib * NB:
                    # keep where i_global - j_global >= 0, else 0
                    nc.gpsimd.affine_select(
                        out=at,
                        in_=at,
                        compare_op=mybir.AluOpType.is_ge,
                        fill=0.0,
                        base=ib * NB - jt * P,
                        pattern=[[1, NB]],
                        channel_multiplier=-1,
                    )
                attnT.append(at)

            # softmax denominators r[i] as [i128, 1], then reciprocal
            recips = []
            for st in range(NB // P):
                it = ib * (NB // P) + st  # global i-tile index
                pr = psum.tile([P, 8], F32, tag="rs", bufs=2, name="pr")
                for jt in range(it + 1):
                    nc.tensor.matmul(
                        pr,
                        attnT[jt][:, st * P : (st + 1) * P],
                        ones,
                        start=(jt == 0),
                        stop=(jt == it),
                    )
                rc = rpool.tile([P, 1], F32, tag="rc", bufs=6, name="rc")
                nc.vector.tensor_copy(out=rc, in_=pr[:, 0:1])
                nc.vector.reciprocal(out=rc, in_=rc)
                recips.append(rc)

            # outT[d, i] = v.T @ attnT
            outT = []
            for dt in range(NT):
                po = psum.tile([P, NB], F32, tag="mm", bufs=6, name="po")
                for jt in range(jt_max):
                    nc.tensor.matmul(
                        po,
                        v[:, jt * D + dt * P : jt * D + (dt + 1) * P],
                        attnT[jt],
                        start=(jt == 0),
                        stop=(jt == jt_max - 1),
                    )
                ot = opool.tile([P, NB], MM_DT, tag="ot", name="ot")
                nc.vector.tensor_copy(out=ot, in_=po)
                outT.append(ot)

            # y[s, e] = outT.T @ woT with softmax normalization fused in
            for st in range(NB // P):
                ysb = ypool.tile([P, D], F32, tag="y", name="ysb")
                row0 = (ib * (NB // P) + st) * P
                for eb in range(NBLK):
                    py = psum.tile([P, NB], F32, tag="mm", bufs=6, name="py")
                    for dt in range(NT):
                        nc.tensor.matmul(
                            py,
                            outT[dt][:, st * P : (st + 1) * P],
                            wo[:, dt * D + eb * NB : dt * D + (eb + 1) * NB],
                            start=(dt == 0),
                            stop=(dt == NT - 1),
                        )
                    nc.vector.tensor_scalar_mul(
                        ysb[:, eb * NB : (eb + 1) * NB], py, recips[st]
                    )
                    # store each half as soon as it is normalized
                    nc.sync.dma_start(
                        y_d[row0 : row0 + P, eb * NB : (eb + 1) * NB],
                        ysb[:, eb * NB : (eb + 1) * NB],
                    )

    nc.finalize()
    return nc


_CACHED_NC = None


def kernel(x, wq, wk, wv, wo, _trace=False):
    global LAST_RESULTS, _CACHED_NC
    assert x.shape == (B, S, D)
    if _CACHED_NC is None:
        _CACHED_NC = _build()
    nc = _CACHED_NC

    wqT = np.ascontiguousarray(wq.T)
    wkT = np.ascontiguousarray(wk.T)
    wvT = np.ascontiguousarray(wv.T)
    woT = np.ascontiguousarray(wo.T)
    in_maps = [
        {
            "xT": np.ascontiguousarray(x[b].T),
            "wqT": wqT,
            "wkT": wkT,
            "wvT": wvT,
            "woT": woT,
        }
        for b in range(N_CORES)
    ]

    res = run_bass_kernel_spmd(
        nc, in_maps, core_ids=list(range(N_CORES)), trace=_trace
    )
    LAST_RESULTS = res
    out = np.stack([res.results[b]["y"] for b in range(N_CORES)], axis=0)
    return out.astype(np.float32, copy=False)


# revision 9
# speedup vs baseline: 1.1335x; 1.0326x over previous
"""Causal single-head self-attention (B=8, S=1024, D=1024, f32) on 8 TRN2 cores.

Sharding: data-parallel over batch (1 batch element per core); the four
d_model^2 weights are replicated. Host-side prep transposes x[b] -> xT [d, s]
and each weight -> wT [d, e] so every on-chip matmul contracts over the
partition dimension with no on-chip transposes.

Per-core dataflow (S=1024 rows of one batch element):
  qT[e,s] = wqT.T @ xT        kT[e,s] = wkT.T @ xT       v[s,e] = xT.T @ wvT
  scoresT[j,i] = kT.T @ qT    (only causal-needed 128x512 blocks)
  attnT = exp(scoresT/32)     (ACT engine, reads PSUM; causal mask via
                               affine_select on diagonal-crossing blocks)
  r[i]  = sum_j attnT[j,i]    (matmuls with a ones vector -> [i,1] PSUM)
  outT[d,i] = v.T @ attnT
  y[s,e] = outT.T @ woT, then y[s,:] *= 1/r[s] fused into the PSUM->SBUF copy.

Matmuls run in float32r (full-rate 4-byte mode, TF32-class rounding). Input
DRAM tensors are declared float32r directly (same bits as f32) so plain HWDGE
DMAs satisfy the BIR verifier's fp32r rounding rule. The first projection
phase is DMA-paced: x arrives as 8 row-slabs interleaved with wq column
chunks, and the first 6 PSUM groups accumulate d-tile-major so matmuls chase
the arriving slabs instead of stalling for the full 8 MB.
"""

import sys

sys.path.insert(0, "/opt/trn_rl_repo")

from contextlib import ExitStack

import numpy as np

import concourse.bass as bass
from concourse import bacc
import concourse.mybir as mybir
import concourse.tile as tile
from concourse.tile import add_dep_helper
from concourse.bass_utils import run_bass_kernel_spmd

B, S, D = 8, 1024, 1024
P = 128          # partition / stationary tile size
NB = 512         # moving-operand block (max for 4-byte dtypes, = 1 PSUM bank)
NT = S // P      # 8 tiles of 128 along s/d/e/j
NBLK = S // NB   # 2 blocks of 512 along s/i/e
SCALE = 1.0 / np.sqrt(float(D))

F32 = mybir.dt.float32
MM_DT = mybir.dt.float32r  # fp32r: full-rate (1 cyc/row) matmul at N>=256

N_CORES = 8

LAST_RESULTS = None  # BassKernelResults of the most recent run (for test.py)


def _build():
    nc = bacc.Bacc("TRN2", target_bir_lowering=False, debug=False)

    xT_d = nc.dram_tensor("xT", [D, S], MM_DT, kind="ExternalInput").ap()
    wqT_d = nc.dram_tensor("wqT", [D, D], MM_DT, kind="ExternalInput").ap()
    wkT_d = nc.dram_tensor("wkT", [D, D], MM_DT, kind="ExternalInput").ap()
    wvT_d = nc.dram_tensor("wvT", [D, D], MM_DT, kind="ExternalInput").ap()
    woT_d = nc.dram_tensor("woT", [D, D], MM_DT, kind="ExternalInput").ap()
    y_d = nc.dram_tensor("y", [S, D], F32, kind="ExternalOutput").ap()

    # SBUF layout of a transposed 1024x1024 matrix: big tile [128, 8192] where
    # column range t*1024..(t+1)*1024 holds DRAM rows t*128..(t+1)*128.
    def slab_load(sbuf_tile, dram_ap, t):
        # one row-slab: DRAM rows t*128..(t+1)*128 (512 KB contiguous)
        return nc.sync.dma_start(
            sbuf_tile[:, t * S : (t + 1) * S],
            dram_ap[t * P : (t + 1) * P, :],
        )

    def chunk_load(sbuf_tile, dram_ap, c):
        # one column-chunk: DRAM cols c*128..(c+1)*128 across all row-slabs
        return nc.sync.dma_start(
            sbuf_tile.rearrange("p (t e) -> p t e", t=NT)[:, :, c * P : (c + 1) * P],
            dram_ap.rearrange("(t p) e -> p t e", p=P)[:, :, c * P : (c + 1) * P],
        )

    with tile.TileContext(nc) as tc, ExitStack() as ctx:
        consts = ctx.enter_context(tc.tile_pool(name="consts", bufs=1))
        ones_f32 = consts.tile([P, 8], F32)
        nc.gpsimd.memset(ones_f32, 1.0)
        ones = consts.tile([P, 8], MM_DT)
        nc.vector.tensor_copy(out=ones, in_=ones_f32)
        zbias = consts.tile([P, 1], F32)
        nc.gpsimd.memset(zbias, 0.0)

        psum = ctx.enter_context(tc.tile_pool(name="psum", bufs=6, space="PSUM"))

        # Two weight slots; wv reuses wq's slot, wo reuses wk's (WAR deps make
        # the DMAs wait for the previous phase's matmuls automatically).
        wpool = ctx.enter_context(tc.tile_pool(name="wpool", bufs=2))
        qpool = ctx.enter_context(tc.tile_pool(name="qpool", bufs=1))
        kpool = ctx.enter_context(tc.tile_pool(name="kpool", bufs=1))
        vpool = ctx.enter_context(tc.tile_pool(name="vpool", bufs=1))

        qT = qpool.tile([P, NT * S], MM_DT, name="qT")
        kT = kpool.tile([P, NT * S], MM_DT, name="kT")
        v = vpool.tile([P, NT * S], MM_DT, name="v")

        wq = wpool.tile([P, NT * D], MM_DT, tag="w", name="wq")
        wk = wpool.tile([P, NT * D], MM_DT, tag="w", name="wk")

        with tc.tile_pool(name="xpool", bufs=1) as xpool:
            xsb = xpool.tile([P, NT * S], MM_DT, name="xsb")

            # Load order: wq chunks for the 8 dt-major phase-0 groups first,
            # then all of x (every group needs every x slab), then the wq tail
            # (chunks 4-7) which overlaps the phase-0 compute.
            for c in range(4):
                chunk_load(wq, wqT_d, c)
            for t in range(NT):
                slab_load(xsb, xT_d, t)
            for c in range(4, NT):
                chunk_load(wq, wqT_d, c)

            def mm_q(pt, et, sb, dt):
                nc.tensor.matmul(
                    pt,
                    wq[:, dt * D + et * P : dt * D + (et + 1) * P],
                    xsb[:, dt * S + sb * NB : dt * S + (sb + 1) * NB],
                    start=(dt == 0),
                    stop=(dt == NT - 1),
                )

            q_copies = {}  # (et, sb) -> copy instruction (for wk prefetch deps)

            def q_copy(pt, et, sb):
                inst = nc.vector.tensor_copy(
                    out=qT[:, et * S + sb * NB : et * S + (sb + 1) * NB],
                    in_=pt,
                )
                q_copies[(et, sb)] = inst
                return inst

            # Phase 0 of P_q: 6 PSUM groups accumulated d-tile-major so the
            # matmul stream follows the arriving x slabs.
            groups = [(et, sb) for et in range(4) for sb in range(NBLK)]
            pts = {}
            for g in groups:
                pts[g] = psum.tile([P, NB], F32, tag="mm", bufs=8, name="pt")
            for dt in range(NT):
                for (et, sb) in groups:
                    mm_q(pts[(et, sb)], et, sb, dt)
            for (et, sb) in groups:
                q_copy(pts[(et, sb)], et, sb)

            # Remaining e-tiles of P_q, standard order.
            for et in range(4, NT):
                for sb in range(NBLK):
                    pt = psum.tile([P, NB], F32, tag="mm", bufs=8, name="pt")
                    for dt in range(NT):
                        mm_q(pt, et, sb, dt)
                    q_copy(pt, et, sb)

            # wk slabs prefetch spread across P_q so they don't steal DMA
            # bandwidth from the x/wq ramp.
            for t in range(NT):
                dma = slab_load(wk, wkT_d, t)
                anchor = q_copies.get((min(1 + t // 2, NT - 1), t % 2))
                if anchor is not None:
                    add_dep_helper(dma.ins, anchor.ins, reason="wk prefetch pacing")

            # P_k: kT[e, s], all inputs resident by now.
            for et in range(NT):
                for sb in range(NBLK):
                    pt = psum.tile([P, NB], F32, tag="mm", bufs=8, name="pt")
                    for dt in range(NT):
                        nc.tensor.matmul(
                            pt,
                            wk[:, dt * D + et * P : dt * D + (et + 1) * P],
                            xsb[:, dt * S + sb * NB : dt * S + (sb + 1) * NB],
                            start=(dt == 0),
                            stop=(dt == NT - 1),
                        )
                    nc.vector.tensor_copy(
                        out=kT[:, et * S + sb * NB : et * S + (sb + 1) * NB],
                        in_=pt,
                    )

            wv = wpool.tile([P, NT * D], MM_DT, tag="w", name="wv")
            for t in range(NT):
                slab_load(wv, wvT_d, t)

            # P_v: v[s, e] natural: stationary xT[d, s128], moving wvT[d, e512]
            for st in range(NT):
                for eb in range(NBLK):
                    pt = psum.tile([P, NB], F32, tag="mm", bufs=8, name="pt")
                    for dt in range(NT):
                        nc.tensor.matmul(
                            pt,
                            xsb[:, dt * S + st * P : dt * S + (st + 1) * P],
                            wv[:, dt * D + eb * NB : dt * D + (eb + 1) * NB],
                            start=(dt == 0),
                            stop=(dt == NT - 1),
                        )
                    nc.vector.tensor_copy(
                        out=v[:, st * D + eb * NB : st * D + (eb + 1) * NB],
                        in_=pt,
                    )

        wo = wpool.tile([P, NT * D], MM_DT, tag="w", name="wo")
        for t in range(NT):
            slab_load(wo, woT_d, t)

        apool = ctx.enter_context(tc.tile_pool(name="apool", bufs=10))
        opool = ctx.enter_context(tc.tile_pool(name="opool", bufs=8))
        ypool = ctx.enter_context(tc.tile_pool(name="ypool", bufs=2))
        rpool = ctx.enter_context(tc.tile_pool(name="rpool", bufs=6))

        for ib in range(NBLK):
            jt_max = (ib + 1) * (NB // P)  # causal: j-tiles 0..jt_max-1

            # scoresT[j, i] -> exp -> attnT tiles in SBUF
            attnT = []
            for jt in range(jt_max):
                ps = psum.tile([P, NB], F32, tag="mm", bufs=8, name="ps")
                for et in range(NT):
                    nc.tensor.matmul(
                        ps,
                        kT[:, et * S + jt * P : et * S + (jt + 1) * P],
                        qT[:, et * S + ib * NB : et * S + (ib + 1) * NB],
                        start=(et == 0),
                        stop=(et == NT - 1),
                    )
                at = apool.tile([P, NB], MM_DT, tag="attn", name="at")
                nc.scalar.activation(
                    out=at,
                    in_=ps,
                    func=mybir.ActivationFunctionType.Exp,
                    bias=zbias,
                    scale=SCALE,
                )
                # blocks fully below the diagonal need no mask
                if jt * P + P - 1 > ib * NB:
                    # keep where i_global - j_global >= 0, else 0
                    nc.gpsimd.affine_select(
                        out=at,
                        in_=at,
                        compare_op=mybir.AluOpType.is_ge,
                        fill=0.0,
                        base=ib * NB - jt * P,
                        pattern=[[1, NB]],
                        channel_multiplier=-1,
                    )
                attnT.append(at)

            # softmax denominators r[i] as [i128, 1], then reciprocal
            recips = []
            for st in range(NB // P):
                it = ib * (NB // P) + st  # global i-tile index
                pr = psum.tile([P, 8], F32, tag="mm", bufs=8, name="pr")
                for jt in range(it + 1):
                    nc.tensor.matmul(
                        pr,
                        attnT[jt][:, st * P : (st + 1) * P],
                        ones,
                        start=(jt == 0),
                        stop=(jt == it),
                    )
                rc = rpool.tile([P, 1], F32, tag="rc", bufs=6, name="rc")
                nc.vector.tensor_copy(out=rc, in_=pr[:, 0:1])
                nc.vector.reciprocal(out=rc, in_=rc)
                recips.append(rc)

            # outT[d, i] = v.T @ attnT
            outT = []
            for dt in range(NT):
                po = psum.tile([P, NB], F32, tag="mm", bufs=8, name="po")
                for jt in range(jt_max):
                    nc.tensor.matmul(
                        po,
                        v[:, jt * D + dt * P : jt * D + (dt + 1) * P],
                        attnT[jt],
                        start=(jt == 0),
                        stop=(jt == jt_max - 1),
                    )
                ot = opool.tile([P, NB], MM_DT, tag="ot", name="ot")
                nc.vector.tensor_copy(out=ot, in_=po)
                outT.append(ot)

            # y[s, e] = outT.T @ woT with softmax normalization fused in
            for st in range(NB // P):
                ysb = ypool.tile([P, D], F32, tag="y", name="ysb")
                row0 = (ib * (NB // P) + st) * P
                for eb in range(NBLK):
                    py = psum.tile([P, NB], F32, tag="mm", bufs=8, name="py")
                    for dt in range(NT):
                        nc.tensor.matmul(
                            py,
                            outT[dt][:, st * P : (st + 1) * P],
                            wo[:, dt * D + eb * NB : dt * D + (eb + 1) * NB],
                            start=(dt == 0),
                            stop=(dt == NT - 1),
                        )
                    nc.vector.tensor_scalar_mul(
                        ysb[:, eb * NB : (eb + 1) * NB], py, recips[st]
                    )
                    # store each half as soon as it is normalized
                    nc.sync.dma_start(
                        y_d[row0 : row0 + P, eb * NB : (eb + 1) * NB],
                        ysb[:, eb * NB : (eb + 1) * NB],
                    )

    nc.finalize()
    return nc


_CACHED_NC = None


def kernel(x, wq, wk, wv, wo, _trace=False):
    global LAST_RESULTS, _CACHED_NC
    assert x.shape == (B, S, D)
    if _CACHED_NC is None:
        _CACHED_NC = _build()
    nc = _CACHED_NC

    wqT = np.ascontiguousarray(wq.T)
    wkT = np.ascontiguousarray(wk.T)
    wvT = np.ascontiguousarray(wv.T)
    woT = np.ascontiguousarray(wo.T)
    in_maps = [
        {
            "xT": np.ascontiguousarray(x[b].T),
            "wqT": wqT,
            "wkT": wkT,
            "wvT": wvT,
            "woT": woT,
        }
        for b in range(N_CORES)
    ]

    res = run_bass_kernel_spmd(
        nc, in_maps, core_ids=list(range(N_CORES)), trace=_trace
    )
    LAST_RESULTS = res
    out = np.stack([res.results[b]["y"] for b in range(N_CORES)], axis=0)
    return out.astype(np.float32, copy=False)
